# revision 21
# baseline (speedup 1.0000x reference)
"""Linear-attention (relu, rmsnorm-qk) Trainium2 Bass kernel, 8 NeuronCores.

Sharding: each core owns 1/4 of the tokens of TWO batch elements:
  cores 0-3 -> batches 0 (group g=0) and 1 (g=1)
  cores 4-7 -> batches 2 (g=0) and 3 (g=1)
Within a batch, core q (= core_id % 4) owns tokens [1024*q, 1024*(q+1)).

Per core, per group (1024 tokens = 8 token-tiles of 128):
  phase 1: qkv = x @ W_qkv (fp32r matmuls, x fed pre-transposed from host),
           rmsnorm+relu on q/k, v_ext = [v | 1], per-head-pair
           kv_ext = k^T @ v_ext accumulated in SBUF
  AllReduce(kv_ext) over the 4 cores of the batch (overlaps the other
           group's phase 1)
  phase 2: attn^T = blockdiag(kv)^T-matmuls on q^T, normalizer via a
           folded k_sum column replicated over the head block, divide,
           out = attn @ W_out (+ b_out)

Token tiles are processed in PAIRS sharing the 6 projection PSUM banks
(3 f-chunks per tile) so two tiles' matmuls interleave while W_qkv is
still streaming from HBM at kernel start.
"""

import os
import sys

import numpy as np

for _p in ("/opt/trn_rl_repo",):
    if _p not in sys.path and os.path.isdir(_p):
        sys.path.insert(0, _p)

import concourse.bass as bass
import concourse.mybir as mybir
import concourse.tile as tile
from concourse import bacc
from concourse.bass_utils import run_bass_kernel_spmd
from concourse.masks import make_identity
from contextlib import ExitStack

F32 = mybir.dt.float32
F32R = mybir.dt.float32r
ALU = mybir.AluOpType
ACTF = mybir.ActivationFunctionType

DIM = 1024
HEADS = 16
DHEAD = 64
NPAIR = HEADS // 2          # 8 head pairs
B = 4
N = 4096
TOK = 2048                  # tokens per core (2 groups x 1024)
GTOK = 1024                 # tokens per group
NTG = GTOK // 128           # 8 token tiles per group
EPS_NORM = 1e-6
KVW = 2 * (DHEAD + 1)       # 130: kv_ext width per pair
RG = [[0, 1, 2, 3], [4, 5, 6, 7]]

_CACHE: dict = {}
KV_BF16 = True
BF16 = mybir.dt.bfloat16


def _build(use_bias: bool, use_w: bool, sim_mode: bool = False):
    ndev = 1 if sim_mode else 8
    nc = bacc.Bacc("TRN2", target_bir_lowering=False, debug=False, num_devices=ndev)

    xT_d = nc.dram_tensor("xT", [DIM, TOK], F32R, kind="ExternalInput").ap()
    wqkv_d = nc.dram_tensor("wqkv", [8, 128, 3 * DIM], F32R, kind="ExternalInput").ap()
    wout_d = nc.dram_tensor("wout", [8, 128, DIM], F32R, kind="ExternalInput").ap()
    qn_d = nc.dram_tensor("qn", [128, DIM], F32, kind="ExternalInput").ap()
    kn_d = nc.dram_tensor("kn", [128, DIM], F32, kind="ExternalInput").ap()
    bout_d = nc.dram_tensor("bout", [128, DIM], F32, kind="ExternalInput").ap()
    out_d = nc.dram_tensor("out", [TOK, DIM], F32, kind="ExternalOutput").ap()

    xT_view = xT_d.rearrange("(c p) n -> p c n", p=128)  # [128, 8, TOK]

    with tile.TileContext(nc) as tc:
        with ExitStack() as outer:
            const = outer.enter_context(tc.tile_pool(name="const", bufs=1))
            wpool = outer.enter_context(tc.tile_pool(name="wpool", bufs=1))
            qTpool = outer.enter_context(tc.tile_pool(name="qTpool", bufs=1))
            stats = outer.enter_context(tc.tile_pool(name="stats", bufs=4))
            drampool = outer.enter_context(
                tc.tile_pool(name="dram", bufs=1, space="DRAM")
            )

            ident = const.tile([128, 128], F32, name="ident")
            make_identity(nc, ident[:])
            ident_r = const.tile([128, 128], F32R, name="ident_r")
            nc.vector.tensor_copy(ident_r[:], ident[:])
            eps_sb = const.tile([128, 1], F32, name="eps_sb")
            nc.vector.memset(eps_sb[:], EPS_NORM)
            ones_sb = const.tile([128, 128], F32, name="ones_sb")
            nc.vector.memset(ones_sb[:], 1.0)
            if use_w:
                qn_sb = const.tile([128, DIM], F32, name="qn_sb")
                kn_sb = const.tile([128, DIM], F32, name="kn_sb")
                nc.sync.dma_start(qn_sb[:], qn_d[:])
                nc.sync.dma_start(kn_sb[:], kn_d[:])
            if use_bias:
                bout_sb = const.tile([128, DIM], F32, name="bout_sb")
                nc.sync.dma_start(bout_sb[:], bout_d[:])

            # W_qkv resident, streamed in first-use order on the SP queue
            # (x^T tiles use the ACT queue so they are not stuck behind it).
            w_sb = []
            for c in range(8):
                w = wpool.tile([128, 3 * DIM], F32R, name=f"wq{c}", tag=f"w{c}")
                w_sb.append(w)
            for fs in ((0, 1, 4), (2, 3, 5)):
                for c in range(8):
                    for f in fs:
                        nc.sync.dma_start(
                            w_sb[c][:, f * 512 : (f + 1) * 512],
                            wqkv_d[c, :, f * 512 : (f + 1) * 512],
                        )

            qT = [
                qTpool.tile([128, TOK], F32R, name=f"qT{j}", tag=f"qT{j}")
                for j in range(8)
            ]

            kvprep = []
            with ExitStack() as ph1:
                xTp = ph1.enter_context(tc.tile_pool(name="xTp", bufs=3))
                qkp = ph1.enter_context(tc.tile_pool(name="qkp", bufs=3))
                sqp = ph1.enter_context(tc.tile_pool(name="sqp", bufs=2))
                vp = ph1.enter_context(tc.tile_pool(name="vp", bufs=3))
                psproj = ph1.enter_context(
                    tc.tile_pool(name="psproj", bufs=1, space="PSUM")
                )
                pssmall = ph1.enter_context(
                    tc.tile_pool(name="pssmall", bufs=2, space="PSUM")
                )
                kvpool = ph1.enter_context(tc.tile_pool(name="kvpool", bufs=1))

                for g in range(2):
                    kv_acc = kvpool.tile(
                        [128, NPAIR, KVW], F32, name=f"kvacc{g}", tag="kvacc"
                    )

                    # --- projection, tile pairs interleaved on 6 psum banks.
                    # half A covers f-chunks {0,1,4} (q + v-half0), half B
                    # {2,3,5} (k + v-half1); each half's psum readers are
                    # traced before the other half reuses the banks. ---
                    HA, HB = (0, 1, 4), (2, 3, 5)
                    for pr in range(NTG // 2):
                        pair = (g * NTG + 2 * pr, g * NTG + 2 * pr + 1)
                        xTt, pstab, sqt, s_qt, s_kt, qsbt, ksbt, vht = (
                            {}, {}, {}, {}, {}, {}, {}, {}
                        )
                        for i in pair:
                            xTt[i] = xTp.tile(
                                [128, 8, 128], F32R, name=f"xT_{i}", tag="xT"
                            )
                            nc.scalar.dma_start(
                                xTt[i][:], xT_view[:, :, i * 128 : (i + 1) * 128]
                            )
                            vht[i] = [None, None]

                        def rstat(i, p0, p1):
                            # 1/sqrt(mean(x^2)+eps) over psum chunks p0,p1
                            sq = sqt[i]
                            a0 = stats.tile([128, 1], F32, name=f"a0_{i}_{p0}", tag="a0")
                            a1 = stats.tile([128, 1], F32, name=f"a1_{i}_{p0}", tag="a1")
                            nc.scalar.activation(
                                sq[:], pstab[(i, p0)][:], ACTF.Square, accum_out=a0[:]
                            )
                            nc.scalar.activation(
                                sq[:], pstab[(i, p1)][:], ACTF.Square, accum_out=a1[:]
                            )
                            nc.vector.tensor_add(a1[:], a1[:], a0[:])
                            nc.scalar.activation(
                                a0[:], a1[:], ACTF.Sqrt,
                                bias=eps_sb[:], scale=1.0 / DIM,
                            )
                            s = stats.tile([128, 1], F32, name=f"s_{i}_{p0}", tag=f"s{p0}")
                            nc.vector.reciprocal(s[:], a0[:])
                            return s

                        def scale_qk(i, dst, p0, s_t, w_t, dtype):
                            t = qkp.tile([128, DIM], dtype, name=f"{dst}_{i}", tag="qk")
                            for h in range(2):
                                sl = slice(h * 512, (h + 1) * 512)
                                if use_w:
                                    nc.vector.scalar_tensor_tensor(
                                        out=t[:, sl],
                                        in0=pstab[(i, p0 + h)][:],
                                        scalar=s_t[:],
                                        in1=w_t[:, sl],
                                        op0=ALU.mult,
                                        op1=ALU.mult,
                                    )
                                else:
                                    nc.vector.tensor_scalar_mul(
                                        t[:, sl], pstab[(i, p0 + h)][:], s_t[:]
                                    )
                            return t

                        def vcopy(i, hh, p):
                            v_sb = vp.tile(
                                [128, 8, DHEAD + 1], BF16 if KV_BF16 else F32,
                                name=f"v{i}_{hh}", tag="v",
                            )
                            nc.vector.memset(v_sb[:, :, DHEAD], 1.0)
                            nc.vector.tensor_copy(
                                v_sb[:, :, 0:DHEAD],
                                pstab[(i, p)].rearrange("p (h e) -> p h e", e=DHEAD),
                            )
                            vht[i][hh] = v_sb

                        def proj_half(fs):
                            for i in pair:
                                par = i % 2
                                for fo, f in enumerate(fs):
                                    pstab[(i, f)] = psproj.tile(
                                        [128, 512],
                                        F32,
                                        name=f"ps{i}_{f}",
                                        tag=f"ps{3 * par + fo}",
                                    )
                            for c in range(8):
                                for i in pair:
                                    lhsT = xTt[i][:, c, :]
                                    for f in fs:
                                        nc.tensor.matmul(
                                            pstab[(i, f)][:],
                                            lhsT,
                                            w_sb[c][:, f * 512 : (f + 1) * 512],
                                            start=(c == 0),
                                            stop=(c == 7),
                                        )

                        proj_half(HA)
                        for i in pair:
                            # epilogue A: q scale, v half0
                            sqt[i] = sqp.tile([128, 512], F32, name=f"sq{i}", tag="sq")
                            s_qt[i] = rstat(i, 0, 1)
                            qsbt[i] = scale_qk(
                                i, "q_sb", 0, s_qt[i],
                                qn_sb if use_w else None, F32R,
                            )
                            vcopy(i, 0, 4)

                        proj_half(HB)
                        for i in pair:
                            ti = i - g * NTG
                            t0 = i * 128
                            # epilogue B: k scale+relu, v half1, kv, q^T
                            s_kt[i] = rstat(i, 2, 3)
                            k_sb = scale_qk(
                                i, "k_sb", 2, s_kt[i],
                                kn_sb if use_w else None,
                                BF16 if KV_BF16 else F32,
                            )
                            nc.scalar.activation(k_sb[:], k_sb[:], ACTF.Relu)
                            vcopy(i, 1, 5)

                            for p in range(NPAIR):
                                kvps = pssmall.tile(
                                    [128, KVW], F32, name=f"kv{i}_{p}", tag="small"
                                )
                                nc.tensor.matmul(
                                    kvps[:],
                                    k_sb[:, p * 128 : (p + 1) * 128],
                                    vht[i][p // 4][:, (2 * p) % 8 : (2 * p) % 8 + 2, :],
                                )
                                if ti == 0:
                                    nc.vector.tensor_copy(kv_acc[:, p, :], kvps[:])
                                else:
                                    nc.vector.tensor_add(
                                        kv_acc[:, p, :], kv_acc[:, p, :], kvps[:]
                                    )

                            for j in range(8):
                                trp = pssmall.tile(
                                    [128, 128], F32R, name=f"tr{i}_{j}", tag="small"
                                )
                                nc.tensor.transpose(
                                    trp[:], qsbt[i][:, j * 128 : (j + 1) * 128],
                                    ident_r[:],
                                )
                                nc.vector.tensor_scalar_max(
                                    qT[j][:, t0 : t0 + 128], trp[:], 0.0
                                )

                    arin = drampool.tile(
                        [128, NPAIR, KVW], F32, name=f"arin{g}", tag=f"arin{g}"
                    )
                    nc.sync.dma_start(arin[:], kv_acc[:])
                    arout = drampool.tile(
                        [128, NPAIR, KVW], F32, name=f"arout{g}", tag=f"arout{g}"
                    )
                    if sim_mode:
                        nc.sync.dma_start(arout[:], arin[:])
                    else:
                        nc.gpsimd.collective_compute(
                            "AllReduce",
                            ALU.add,
                            replica_groups=RG,
                            ins=[arin.opt()],
                            outs=[arout.opt()],
                        )

                    kvprep.append(arout)

            # kv_sb / blockdiag / replicated-ksum prep. g0's is traced
            # right after its AllReduce so it overlaps g1's phase 1; g1's
            # is traced inside phase 2 (its shared slots are only free
            # after g0's attention matmuls anyway).
            def kv_prep(g):
                arout = kvprep[g]
                kv_sb = p2p.tile(
                    [128, NPAIR, KVW], F32, name=f"kvsb{g}", tag="kvsb"
                )
                nc.sync.dma_start(kv_sb[:], arout[:])
                bd = p2p.tile([128, NPAIR, 128], F32R, name=f"bd{g}", tag="bd")
                for p in range(NPAIR):
                    nc.vector.tensor_scalar_mul(bd[:, p, :], ones_sb[:], 0.0)
                ksr = p2p.tile(
                    [128, NPAIR, 128], F32R, name=f"ksr{g}", tag="ksr"
                )
                for p in range(NPAIR):
                    nc.vector.tensor_copy(bd[0:64, p, 0:64], kv_sb[0:64, p, 0:64])
                    nc.vector.tensor_copy(
                        bd[64:128, p, 64:128], kv_sb[64:128, p, 65:129]
                    )
                    nc.vector.tensor_scalar_mul(
                        ksr[0:64, p, 0:64], ones_sb[0:64, 0:64],
                        kv_sb[0:64, p, 64:65],
                    )
                    nc.vector.tensor_scalar_mul(
                        ksr[64:128, p, 64:128], ones_sb[64:128, 0:64],
                        kv_sb[64:128, p, 129:130],
                    )
                return kv_sb, bd, ksr

            # ------------- phase 2 -------------
            wout_sb = []
            for c in range(8):
                w = wpool.tile([128, DIM], F32R, name=f"wo{c}", tag=f"w{c}")
                wout_sb.append(w)
            for ff in range(2):
                for c in range(8):
                    nc.sync.dma_start(
                        wout_sb[c][:, ff * 512 : (ff + 1) * 512],
                        wout_d[c, :, ff * 512 : (ff + 1) * 512],
                    )

            with ExitStack() as ph2:
                p2p = ph2.enter_context(tc.tile_pool(name="p2p", bufs=1))
                atp = ph2.enter_context(tc.tile_pool(name="atp", bufs=1))
                recp = ph2.enter_context(tc.tile_pool(name="recp", bufs=2))
                osbp = ph2.enter_context(tc.tile_pool(name="osbp", bufs=2))
                psattn = ph2.enter_context(
                    tc.tile_pool(name="psattn", bufs=3, space="PSUM")
                )
                psnorm = ph2.enter_context(
                    tc.tile_pool(name="psnorm", bufs=2, space="PSUM")
                )
                psout = ph2.enter_context(
                    tc.tile_pool(name="psout", bufs=3, space="PSUM")
                )

                prep0 = kv_prep(0)
                for g in range(2):
                    kv_sb, bd, ksr = prep0 if g == 0 else kv_prep(1)
                    for hc in range(4):
                        cc = 4 * g + hc
                        c0 = cc * 256
                        attnT = [
                            atp.tile(
                                [128, 256], F32R,
                                name=f"at{cc}_{p}", tag=f"at{p}_{cc % 2}",
                            )
                            for p in range(NPAIR)
                        ]
                        for p in range(NPAIR):
                            aps = psattn.tile(
                                [128, 256], F32, name=f"aps{cc}_{p}", tag="aps"
                            )
                            nc.tensor.matmul(
                                aps[:], bd[:, p, :], qT[p][:, c0 : c0 + 256]
                            )
                            nps = psnorm.tile(
                                [128, 256], F32, name=f"nps{cc}_{p}", tag="nps"
                            )
                            nc.tensor.matmul(
                                nps[:], ksr[:, p, :], qT[p][:, c0 : c0 + 256]
                            )
                            rec = recp.tile(
                                [128, 256], F32, name=f"rec{cc}_{p}", tag="rec"
                            )
                            nc.vector.reciprocal(rec[:], nps[:])
                            nc.vector.tensor_tensor(
                                attnT[p][:], aps[:], rec[:], ALU.mult
                            )

                        for tt in range(2):
                            r0 = c0 + tt * 128
                            ops = [
                                psout.tile(
                                    [128, 512], F32, name=f"o{cc}_{tt}_{ff}", tag="ops"
                                )
                                for ff in range(2)
                            ]
                            for j in range(8):
                                lhsT = attnT[j][:, tt * 128 : (tt + 1) * 128]
                                for ff in range(2):
                                    nc.tensor.matmul(
                                        ops[ff][:],
                                        lhsT,
                                        wout_sb[j][:, ff * 512 : (ff + 1) * 512],
                                        start=(j == 0),
                                        stop=(j == 7),
                                    )
                            for ff in range(2):
                                fsl = slice(ff * 512, (ff + 1) * 512)
                                osb = osbp.tile(
                                    [128, 512], F32, name=f"osb{cc}{tt}{ff}", tag="osb"
                                )
                                if use_bias:
                                    nc.vector.tensor_tensor(
                                        osb[:], ops[ff][:], bout_sb[:, fsl], ALU.add
                                    )
                                else:
                                    nc.scalar.copy(osb[:], ops[ff][:])
                                nc.sync.dma_start(out_d[r0 : r0 + 128, fsl], osb[:])

    nc.compile()
    return nc


def _get_nc(use_bias: bool, use_w: bool):
    key = ("nc", use_bias, use_w)
    if key not in _CACHE:
        _CACHE[key] = _build(use_bias, use_w)
    return _CACHE[key]


def make_in_maps(x, W_qkv, qn_w, kn_w, W_out, b_out):
    x = np.asarray(x, dtype=np.float32)
    W_qkv = np.ascontiguousarray(np.asarray(W_qkv, dtype=np.float32)).reshape(
        8, 128, 3 * DIM
    )
    W_out = np.ascontiguousarray(np.asarray(W_out, dtype=np.float32)).reshape(
        8, 128, DIM
    )
    qn = np.ascontiguousarray(
        np.broadcast_to(np.asarray(qn_w, dtype=np.float32).reshape(1, DIM), (128, DIM))
    )
    kn = np.ascontiguousarray(
        np.broadcast_to(np.asarray(kn_w, dtype=np.float32).reshape(1, DIM), (128, DIM))
    )
    bout = np.ascontiguousarray(
        np.broadcast_to(np.asarray(b_out, dtype=np.float32).reshape(1, DIM), (128, DIM))
    )
    in_maps = []
    for c in range(8):
        b0 = 2 * (c // 4)
        q = c % 4
        sl = slice(1024 * q, 1024 * (q + 1))
        xt = np.concatenate(
            [x[b0, sl, :].T, x[b0 + 1, sl, :].T], axis=1
        )  # [1024, 2048]
        in_maps.append(
            {
                "xT": np.ascontiguousarray(xt),
                "wqkv": W_qkv,
                "wout": W_out,
                "qn": qn,
                "kn": kn,
                "bout": bout,
            }
        )
    return in_maps


def assemble(results):
    out = np.empty((B, N, DIM), dtype=np.float32)
    for b in range(B):
        base = 4 * (b // 2)
        g = b % 2
        for q in range(4):
            out[b, 1024 * q : 1024 * (q + 1), :] = results[base + q]["out"][
                1024 * g : 1024 * (g + 1), :
            ]
    return out


def run(in_maps, use_bias, use_w, **kw):
    nc = _get_nc(use_bias, use_w)
    return run_bass_kernel_spmd(nc, in_maps, core_ids=list(range(8)), **kw)


def kernel(x, W_qkv, qn_w, kn_w, W_out, b_out):
    use_bias = bool(np.any(np.asarray(b_out)))
    use_w = not (
        np.all(np.asarray(qn_w) == 1.0) and np.all(np.asarray(kn_w) == 1.0)
    )
    in_maps = make_in_maps(x, W_qkv, qn_w, kn_w, W_out, b_out)
    res = run(in_maps, use_bias, use_w)
    return assemble(res.results)


# revision 23
# speedup vs baseline: 1.0606x; 1.0606x over previous
"""Linear-attention (relu, rmsnorm-qk) Trainium2 Bass kernel, 8 NeuronCores.

Sharding: each core owns 1/4 of the tokens of TWO batch elements:
  cores 0-3 -> batches 0 (group g=0) and 1 (g=1)
  cores 4-7 -> batches 2 (g=0) and 3 (g=1)
Within a batch, core q (= core_id % 4) owns tokens [1024*q, 1024*(q+1)).

Per core, per group (1024 tokens = 8 token-tiles of 128):
  phase 1: qkv = x @ W_qkv (fp32r matmuls, x fed pre-transposed from host),
           rmsnorm+relu on q/k, v_ext = [v | 1], per-head-pair
           kv_ext = k^T @ v_ext accumulated in SBUF
  AllReduce(kv_ext) over the 4 cores of the batch (overlaps the other
           group's phase 1)
  phase 2: attn^T = blockdiag(kv)^T-matmuls on q^T, normalizer via a
           folded k_sum column replicated over the head block, divide,
           out = attn @ W_out (+ b_out)

Token tiles are processed in PAIRS sharing the 6 projection PSUM banks
(3 f-chunks per tile) so two tiles' matmuls interleave while W_qkv is
still streaming from HBM at kernel start.
"""

import os
import sys

import numpy as np

for _p in ("/opt/trn_rl_repo",):
    if _p not in sys.path and os.path.isdir(_p):
        sys.path.insert(0, _p)

import concourse.bass as bass
import concourse.mybir as mybir
import concourse.tile as tile
from concourse import bacc
from concourse.bass_utils import run_bass_kernel_spmd
from concourse.masks import make_identity
from contextlib import ExitStack

F32 = mybir.dt.float32
F32R = mybir.dt.float32r
ALU = mybir.AluOpType
ACTF = mybir.ActivationFunctionType

DIM = 1024
HEADS = 16
DHEAD = 64
NPAIR = HEADS // 2          # 8 head pairs
B = 4
N = 4096
TOK = 2048                  # tokens per core (2 groups x 1024)
GTOK = 1024                 # tokens per group
NTG = GTOK // 128           # 8 token tiles per group
EPS_NORM = 1e-6
KVW = 2 * (DHEAD + 1)       # 130: kv_ext width per pair
RG = [[0, 1, 2, 3], [4, 5, 6, 7]]

_CACHE: dict = {}
KV_BF16 = False
BF16 = mybir.dt.bfloat16


def _build(use_bias: bool, use_w: bool, sim_mode: bool = False):
    ndev = 1 if sim_mode else 8
    nc = bacc.Bacc("TRN2", target_bir_lowering=False, debug=False, num_devices=ndev)

    xT_d = nc.dram_tensor("xT", [DIM, TOK], F32R, kind="ExternalInput").ap()
    wqkv_d = nc.dram_tensor("wqkv", [8, 128, 3 * DIM], F32R, kind="ExternalInput").ap()
    wout_d = nc.dram_tensor("wout", [8, 128, DIM], F32R, kind="ExternalInput").ap()
    qn_d = nc.dram_tensor("qn", [128, DIM], F32, kind="ExternalInput").ap()
    kn_d = nc.dram_tensor("kn", [128, DIM], F32, kind="ExternalInput").ap()
    bout_d = nc.dram_tensor("bout", [128, DIM], F32, kind="ExternalInput").ap()
    out_d = nc.dram_tensor("out", [TOK, DIM], F32, kind="ExternalOutput").ap()

    xT_view = xT_d.rearrange("(c p) n -> p c n", p=128)  # [128, 8, TOK]

    with tile.TileContext(nc) as tc:
        with ExitStack() as outer:
            const = outer.enter_context(tc.tile_pool(name="const", bufs=1))
            wpool = outer.enter_context(tc.tile_pool(name="wpool", bufs=1))
            qTpool = outer.enter_context(tc.tile_pool(name="qTpool", bufs=1))
            stats = outer.enter_context(tc.tile_pool(name="stats", bufs=4))
            drampool = outer.enter_context(
                tc.tile_pool(name="dram", bufs=1, space="DRAM")
            )

            ident = const.tile([128, 128], F32, name="ident")
            make_identity(nc, ident[:])
            ident_r = const.tile([128, 128], F32R, name="ident_r")
            nc.vector.tensor_copy(ident_r[:], ident[:])
            eps_sb = const.tile([128, 1], F32, name="eps_sb")
            nc.vector.memset(eps_sb[:], EPS_NORM)
            ones_sb = const.tile([128, 128], F32, name="ones_sb")
            nc.vector.memset(ones_sb[:], 1.0)
            if use_w:
                qn_sb = const.tile([128, DIM], F32, name="qn_sb")
                kn_sb = const.tile([128, DIM], F32, name="kn_sb")
                nc.sync.dma_start(qn_sb[:], qn_d[:])
                nc.sync.dma_start(kn_sb[:], kn_d[:])
            if use_bias:
                bout_sb = const.tile([128, DIM], F32, name="bout_sb")
                nc.sync.dma_start(bout_sb[:], bout_d[:])

            # W_qkv resident, streamed in first-use order on the SP queue
            # (x^T tiles use the ACT queue so they are not stuck behind it).
            w_sb = []
            for c in range(8):
                w = wpool.tile([128, 3 * DIM], F32R, name=f"wq{c}", tag=f"w{c}")
                w_sb.append(w)
            for fs in ((0, 1, 4), (2, 3, 5)):
                for c in range(8):
                    for f in fs:
                        nc.sync.dma_start(
                            w_sb[c][:, f * 512 : (f + 1) * 512],
                            wqkv_d[c, :, f * 512 : (f + 1) * 512],
                        )

            qT = [
                qTpool.tile([128, TOK], F32R, name=f"qT{j}", tag=f"qT{j}")
                for j in range(8)
            ]

            kvprep = []
            with ExitStack() as ph1:
                xTp = ph1.enter_context(tc.tile_pool(name="xTp", bufs=3))
                qkp = ph1.enter_context(tc.tile_pool(name="qkp", bufs=3))
                sqp = ph1.enter_context(tc.tile_pool(name="sqp", bufs=2))
                vp = ph1.enter_context(tc.tile_pool(name="vp", bufs=3))
                psproj = ph1.enter_context(
                    tc.tile_pool(name="psproj", bufs=1, space="PSUM")
                )
                pssmall = ph1.enter_context(
                    tc.tile_pool(name="pssmall", bufs=2, space="PSUM")
                )
                kvpool = ph1.enter_context(tc.tile_pool(name="kvpool", bufs=1))

                for g in range(2):
                    kv_acc = kvpool.tile(
                        [128, NPAIR, KVW], F32, name=f"kvacc{g}", tag="kvacc"
                    )

                    # --- projection, tile pairs interleaved on 6 psum banks.
                    # half A covers f-chunks {0,1,4} (q + v-half0), half B
                    # {2,3,5} (k + v-half1); each half's psum readers are
                    # traced before the other half reuses the banks. ---
                    HA, HB = (0, 1, 4), (2, 3, 5)
                    for pr in range(NTG // 2):
                        pair = (g * NTG + 2 * pr, g * NTG + 2 * pr + 1)
                        xTt, pstab, sqt, s_qt, s_kt, qsbt, ksbt, vht = (
                            {}, {}, {}, {}, {}, {}, {}, {}
                        )
                        for i in pair:
                            xTt[i] = xTp.tile(
                                [128, 8, 128], F32R, name=f"xT_{i}", tag="xT"
                            )
                            nc.scalar.dma_start(
                                xTt[i][:], xT_view[:, :, i * 128 : (i + 1) * 128]
                            )
                            vht[i] = [None, None]

                        def rstat(i, p0, p1):
                            # 1/sqrt(mean(x^2)+eps) over psum chunks p0,p1
                            sq = sqt[i]
                            a0 = stats.tile([128, 1], F32, name=f"a0_{i}_{p0}", tag="a0")
                            a1 = stats.tile([128, 1], F32, name=f"a1_{i}_{p0}", tag="a1")
                            nc.scalar.activation(
                                sq[:], pstab[(i, p0)][:], ACTF.Square, accum_out=a0[:]
                            )
                            nc.scalar.activation(
                                sq[:], pstab[(i, p1)][:], ACTF.Square, accum_out=a1[:]
                            )
                            nc.vector.tensor_add(a1[:], a1[:], a0[:])
                            nc.scalar.activation(
                                a0[:], a1[:], ACTF.Sqrt,
                                bias=eps_sb[:], scale=1.0 / DIM,
                            )
                            s = stats.tile([128, 1], F32, name=f"s_{i}_{p0}", tag=f"s{p0}")
                            nc.vector.reciprocal(s[:], a0[:])
                            return s

                        def scale_qk(i, dst, p0, s_t, w_t, dtype):
                            t = qkp.tile([128, DIM], dtype, name=f"{dst}_{i}", tag="qk")
                            for h in range(2):
                                sl = slice(h * 512, (h + 1) * 512)
                                if use_w:
                                    nc.vector.scalar_tensor_tensor(
                                        out=t[:, sl],
                                        in0=pstab[(i, p0 + h)][:],
                                        scalar=s_t[:],
                                        in1=w_t[:, sl],
                                        op0=ALU.mult,
                                        op1=ALU.mult,
                                    )
                                else:
                                    nc.vector.tensor_scalar_mul(
                                        t[:, sl], pstab[(i, p0 + h)][:], s_t[:]
                                    )
                            return t

                        def vcopy(i, hh, p):
                            v_sb = vp.tile(
                                [128, 8, DHEAD + 1], F32R,
                                name=f"v{i}_{hh}", tag="v",
                            )
                            nc.vector.tensor_scalar_mul(
                                v_sb[:, :, DHEAD], ones_sb[:, 0:8], 1.0
                            )
                            nc.vector.tensor_copy(
                                v_sb[:, :, 0:DHEAD],
                                pstab[(i, p)].rearrange("p (h e) -> p h e", e=DHEAD),
                            )
                            vht[i][hh] = v_sb

                        def proj_half(fs):
                            for i in pair:
                                par = i % 2
                                for fo, f in enumerate(fs):
                                    pstab[(i, f)] = psproj.tile(
                                        [128, 512],
                                        F32,
                                        name=f"ps{i}_{f}",
                                        tag=f"ps{3 * par + fo}",
                                    )
                            for c in range(8):
                                for i in pair:
                                    lhsT = xTt[i][:, c, :]
                                    for f in fs:
                                        nc.tensor.matmul(
                                            pstab[(i, f)][:],
                                            lhsT,
                                            w_sb[c][:, f * 512 : (f + 1) * 512],
                                            start=(c == 0),
                                            stop=(c == 7),
                                        )

                        proj_half(HA)
                        for i in pair:
                            # epilogue A: q scale, v half0
                            sqt[i] = sqp.tile([128, 512], F32, name=f"sq{i}", tag="sq")
                            s_qt[i] = rstat(i, 0, 1)
                            qsbt[i] = scale_qk(
                                i, "q_sb", 0, s_qt[i],
                                qn_sb if use_w else None, F32R,
                            )
                            vcopy(i, 0, 4)

                        proj_half(HB)
                        for i in pair:
                            ti = i - g * NTG
                            t0 = i * 128
                            # epilogue B: k scale+relu, v half1, kv, q^T
                            s_kt[i] = rstat(i, 2, 3)
                            k_sb = scale_qk(
                                i, "k_sb", 2, s_kt[i],
                                kn_sb if use_w else None, F32R,
                            )
                            nc.scalar.activation(k_sb[:], k_sb[:], ACTF.Relu)
                            vcopy(i, 1, 5)

                            for pb in range(NPAIR // 2):
                                rhs = vht[i][pb // 2][
                                    :, (4 * pb) % 8 : (4 * pb) % 8 + 4, :
                                ]
                                for sub in range(2):
                                    p = 2 * pb + sub
                                    kvps = pssmall.tile(
                                        [128, 2 * KVW], F32,
                                        name=f"kv{i}_{p}", tag="small",
                                    )
                                    nc.tensor.matmul(
                                        kvps[:],
                                        k_sb[:, p * 128 : (p + 1) * 128],
                                        rhs,
                                    )
                                    use = kvps[:, sub * KVW : (sub + 1) * KVW]
                                    if ti == 0:
                                        nc.vector.tensor_copy(kv_acc[:, p, :], use)
                                    else:
                                        nc.vector.tensor_add(
                                            kv_acc[:, p, :], kv_acc[:, p, :], use
                                        )

                            for j in range(8):
                                trp = pssmall.tile(
                                    [128, 128], F32R, name=f"tr{i}_{j}", tag="small"
                                )
                                nc.tensor.transpose(
                                    trp[:], qsbt[i][:, j * 128 : (j + 1) * 128],
                                    ident_r[:],
                                )
                                nc.vector.tensor_scalar_max(
                                    qT[j][:, t0 : t0 + 128], trp[:], 0.0
                                )

                    arin = drampool.tile(
                        [128, NPAIR, KVW], F32, name=f"arin{g}", tag=f"arin{g}"
                    )
                    nc.sync.dma_start(arin[:], kv_acc[:])
                    arout = drampool.tile(
                        [128, NPAIR, KVW], F32, name=f"arout{g}", tag=f"arout{g}"
                    )
                    if sim_mode:
                        nc.sync.dma_start(arout[:], arin[:])
                    else:
                        nc.gpsimd.collective_compute(
                            "AllReduce",
                            ALU.add,
                            replica_groups=RG,
                            ins=[arin.opt()],
                            outs=[arout.opt()],
                        )

                    kvprep.append(arout)

            # kv_sb / blockdiag / replicated-ksum prep. g0's is traced
            # right after its AllReduce so it overlaps g1's phase 1; g1's
            # is traced inside phase 2 (its shared slots are only free
            # after g0's attention matmuls anyway).
            def kv_prep(g):
                arout = kvprep[g]
                kv_sb = p2p.tile(
                    [128, NPAIR, KVW], F32, name=f"kvsb{g}", tag="kvsb"
                )
                nc.sync.dma_start(kv_sb[:], arout[:])
                bd = p2p.tile([128, NPAIR, 128], F32R, name=f"bd{g}", tag="bd")
                for p in range(NPAIR):
                    nc.vector.tensor_scalar_mul(bd[:, p, :], ones_sb[:], 0.0)
                ksr = p2p.tile(
                    [128, NPAIR, 128], F32R, name=f"ksr{g}", tag="ksr"
                )
                for p in range(NPAIR):
                    nc.vector.tensor_copy(bd[0:64, p, 0:64], kv_sb[0:64, p, 0:64])
                    nc.vector.tensor_copy(
                        bd[64:128, p, 64:128], kv_sb[64:128, p, 65:129]
                    )
                    nc.vector.tensor_scalar_mul(
                        ksr[0:64, p, 0:64], ones_sb[0:64, 0:64],
                        kv_sb[0:64, p, 64:65],
                    )
                    nc.vector.tensor_scalar_mul(
                        ksr[64:128, p, 64:128], ones_sb[64:128, 0:64],
                        kv_sb[64:128, p, 129:130],
                    )
                return kv_sb, bd, ksr

            # ------------- phase 2 -------------
            wout_sb = []
            for c in range(8):
                w = wpool.tile([128, DIM], F32R, name=f"wo{c}", tag=f"w{c}")
                wout_sb.append(w)
            for ff in range(2):
                for c in range(8):
                    nc.sync.dma_start(
                        wout_sb[c][:, ff * 512 : (ff + 1) * 512],
                        wout_d[c, :, ff * 512 : (ff + 1) * 512],
                    )

            with ExitStack() as ph2:
                p2p = ph2.enter_context(tc.tile_pool(name="p2p", bufs=1))
                atp = ph2.enter_context(tc.tile_pool(name="atp", bufs=1))
                recp = ph2.enter_context(tc.tile_pool(name="recp", bufs=2))
                osbp = ph2.enter_context(tc.tile_pool(name="osbp", bufs=2))
                psattn = ph2.enter_context(
                    tc.tile_pool(name="psattn", bufs=3, space="PSUM")
                )
                psnorm = ph2.enter_context(
                    tc.tile_pool(name="psnorm", bufs=2, space="PSUM")
                )
                psout = ph2.enter_context(
                    tc.tile_pool(name="psout", bufs=3, space="PSUM")
                )

                prep0 = kv_prep(0)
                for g in range(2):
                    kv_sb, bd, ksr = prep0 if g == 0 else kv_prep(1)
                    for hc in range(4):
                        cc = 4 * g + hc
                        c0 = cc * 256
                        attnT = [
                            atp.tile(
                                [128, 256], F32R,
                                name=f"at{cc}_{p}", tag=f"at{p}_{cc % 2}",
                            )
                            for p in range(NPAIR)
                        ]
                        for p in range(NPAIR):
                            aps = psattn.tile(
                                [128, 256], F32, name=f"aps{cc}_{p}", tag="aps"
                            )
                            nc.tensor.matmul(
                                aps[:], bd[:, p, :], qT[p][:, c0 : c0 + 256]
                            )
                            nps = psnorm.tile(
                                [128, 256], F32, name=f"nps{cc}_{p}", tag="nps"
                            )
                            nc.tensor.matmul(
                                nps[:], ksr[:, p, :], qT[p][:, c0 : c0 + 256]
                            )
                            rec = recp.tile(
                                [128, 256], F32, name=f"rec{cc}_{p}", tag="rec"
                            )
                            nc.vector.reciprocal(rec[:], nps[:])
                            nc.vector.tensor_tensor(
                                attnT[p][:], aps[:], rec[:], ALU.mult
                            )

                        for tt in range(2):
                            r0 = c0 + tt * 128
                            ops = [
                                psout.tile(
                                    [128, 512], F32, name=f"o{cc}_{tt}_{ff}", tag="ops"
                                )
                                for ff in range(2)
                            ]
                            for j in range(8):
                                lhsT = attnT[j][:, tt * 128 : (tt + 1) * 128]
                                for ff in range(2):
                                    nc.tensor.matmul(
                                        ops[ff][:],
                                        lhsT,
                                        wout_sb[j][:, ff * 512 : (ff + 1) * 512],
                                        start=(j == 0),
                                        stop=(j == 7),
                                    )
                            for ff in range(2):
                                fsl = slice(ff * 512, (ff + 1) * 512)
                                osb = osbp.tile(
                                    [128, 512], F32, name=f"osb{cc}{tt}{ff}", tag="osb"
                                )
                                if use_bias:
                                    nc.vector.tensor_tensor(
                                        osb[:], ops[ff][:], bout_sb[:, fsl], ALU.add
                                    )
                                else:
                                    nc.scalar.copy(osb[:], ops[ff][:])
                                nc.sync.dma_start(out_d[r0 : r0 + 128, fsl], osb[:])

    nc.compile()
    return nc


def _get_nc(use_bias: bool, use_w: bool):
    key = ("nc", use_bias, use_w)
    if key not in _CACHE:
        _CACHE[key] = _build(use_bias, use_w)
    return _CACHE[key]


def make_in_maps(x, W_qkv, qn_w, kn_w, W_out, b_out):
    x = np.asarray(x, dtype=np.float32)
    W_qkv = np.ascontiguousarray(np.asarray(W_qkv, dtype=np.float32)).reshape(
        8, 128, 3 * DIM
    )
    W_out = np.ascontiguousarray(np.asarray(W_out, dtype=np.float32)).reshape(
        8, 128, DIM
    )
    qn = np.ascontiguousarray(
        np.broadcast_to(np.asarray(qn_w, dtype=np.float32).reshape(1, DIM), (128, DIM))
    )
    kn = np.ascontiguousarray(
        np.broadcast_to(np.asarray(kn_w, dtype=np.float32).reshape(1, DIM), (128, DIM))
    )
    bout = np.ascontiguousarray(
        np.broadcast_to(np.asarray(b_out, dtype=np.float32).reshape(1, DIM), (128, DIM))
    )
    in_maps = []
    for c in range(8):
        b0 = 2 * (c // 4)
        q = c % 4
        sl = slice(1024 * q, 1024 * (q + 1))
        xt = np.concatenate(
            [x[b0, sl, :].T, x[b0 + 1, sl, :].T], axis=1
        )  # [1024, 2048]
        in_maps.append(
            {
                "xT": np.ascontiguousarray(xt),
                "wqkv": W_qkv,
                "wout": W_out,
                "qn": qn,
                "kn": kn,
                "bout": bout,
            }
        )
    return in_maps


def assemble(results):
    out = np.empty((B, N, DIM), dtype=np.float32)
    for b in range(B):
        base = 4 * (b // 2)
        g = b % 2
        for q in range(4):
            out[b, 1024 * q : 1024 * (q + 1), :] = results[base + q]["out"][
                1024 * g : 1024 * (g + 1), :
            ]
    return out


def run(in_maps, use_bias, use_w, **kw):
    nc = _get_nc(use_bias, use_w)
    return run_bass_kernel_spmd(nc, in_maps, core_ids=list(range(8)), **kw)


def kernel(x, W_qkv, qn_w, kn_w, W_out, b_out):
    use_bias = bool(np.any(np.asarray(b_out)))
    use_w = not (
        np.all(np.asarray(qn_w) == 1.0) and np.all(np.asarray(kn_w) == 1.0)
    )
    in_maps = make_in_maps(x, W_qkv, qn_w, kn_w, W_out, b_out)
    res = run(in_maps, use_bias, use_w)
    return assemble(res.results)
